# revision 5
# baseline (speedup 1.0000x reference)
"""Trainium2 Bass kernel for nn_Attention_40381282517350.

Reference computation (per batch b):
    qkv = x @ W_qkv ; q,k,v = split(qkv)
    z   = gelu(x @ W_gate + b_gate)           (exact erf gelu)
    pe  = depthwise_conv3x3(q as [C,64,64] image) + pe_b
    attn = softmax(q @ k.T / 16)
    out = (attn @ v + pe) * z
    y   = out @ W_proj

Sharding: 8 cores = (batch b in 0..3) x (sequence half h in 0..1).
Each core receives the full x[b] (to compute k/v) plus a zero-padded
query window xq = x[b, s-64:e+64] (s = h*2048, e = s+2048), and
produces y rows [s, e) of batch b.

On-core layout strategy (transpose-minimal):
  x^T, xq^T via PE transposes -> k^T = W_k.T @ x^T, q^T = W_q.T @ xq^T,
  z^T = gelu(W_g.T @ xq^T + bg), v natural = x^T.T @ W_v (x^T chunks as
  stationary).  Attention computes S^T = K Q^T tiles [m=128, nq=512]
  (softmax denominators via a ones-row matmul; no max-subtraction is
  needed: |logits| <~ 1 for this data distribution), exp on ACT, then
  O^T[c,nq] accumulates with V chunks as the matmul stationary so the
  exp output is consumed in-place with no transposes.  pe conv runs as
  9 fused scalar_tensor_tensor taps on sliced [128,34,64] views of q^T.
  merge = (O^T/sums + pe) * z^T happens in [c, n] layout, which is
  exactly the stationary layout proj needs to emit y natural.

Big matmuls run in float32r (tf32-like, full PE speed at N>=256);
everything else fp32.
"""

import numpy as np

import concourse.bass as bass
import concourse.tile as tile
from concourse import bacc, mybir
from concourse.bass_utils import run_bass_kernel_spmd

dt = mybir.dt
f32 = dt.float32
f32r = dt.float32r

B, N, C = 4, 4096, 256
WS = 64            # image side (N = WS*WS)
HALF = N // 2      # rows per core
HALO = WS          # one image row of halo for the conv
NQW = HALF + 2 * HALO   # 2176 query-window rows (= 34 image rows)
N_CORES = 8
SCALE = C ** -0.5  # 1/16
NG = HALF // 512   # 4 q-groups of 512 rows
NM = N // 128      # 32 m-tiles

_nc_cache = None


def _r(ap):
    return ap


def _emit(nc, tc, aps):
    X, XQ, WQ, WK, WV, WG, BG, WP, PW, PB, IDN, Y = aps

    with (
        tc.tile_pool(name="wpool", bufs=1) as wp_pool,
        tc.tile_pool(name="persist", bufs=1) as pp,
        tc.tile_pool(name="psum", bufs=1, space="PSUM") as ps,
    ):
        # ---- weights to SBUF ----
        def load_w(name, src):
            t = [wp_pool.tile([128, 256], f32r, tag=f"{name}{ci}", name=f"{name}{ci}") for ci in range(2)]
            for ci in range(2):
                stg = wp_pool.tile([128, 256], f32, tag="wstg", name="wstg", bufs=2)
                nc.sync.dma_start(stg[:, :], src[128 * ci:128 * (ci + 1), :])
                nc.vector.tensor_copy(t[ci][:, :], stg[:, :])
            return t

        wq_sb = load_w("wq", WQ)
        wk_sb = load_w("wk", WK)
        wv_sb = load_w("wv", WV)
        wg_sb = load_w("wg", WG)
        wp_sb = load_w("wp", WP)
        bg_sb = wp_pool.tile([128, 2], f32, tag="bg", name="bg_sb")
        pb_sb = wp_pool.tile([128, 2], f32, tag="pb", name="pb_sb")
        pw_sb = wp_pool.tile([128, 2, 9], f32, tag="pw", name="pw_sb")
        for ci in range(2):
            nc.sync.dma_start(bg_sb[:, ci:ci + 1], BG[128 * ci:128 * (ci + 1), :])
            nc.sync.dma_start(pb_sb[:, ci:ci + 1], PB[128 * ci:128 * (ci + 1), :])
            nc.sync.dma_start(pw_sb[:, ci, :], PW[128 * ci:128 * (ci + 1), :])
        ident = wp_pool.tile([128, 128], f32, tag="ident", name="ident")
        nc.sync.dma_start(ident[:, :], IDN[:, :])
        ones_f = wp_pool.tile([128, 1], f32, tag="ones_f", name="ones_f")
        nc.vector.memset(ones_f[:, :], 1.0)
        ones_m = wp_pool.tile([128, 1], f32r, tag="ones_m", name="ones_m")   # sums stationary
        nc.vector.tensor_copy(ones_m[:, :], ones_f[:, :])
        ones_1 = wp_pool.tile([1, 128], f32, tag="ones_1", name="ones_1")   # recip broadcast
        nc.vector.memset(ones_1[:, :], 1.0)

        # ---- persistent activations ----
        kT = [pp.tile([128, N], f32r, tag=f"kT{ci}", name=f"kT{ci}") for ci in range(2)]
        qT = [pp.tile([128, NQW], f32r, tag=f"qT{ci}", name=f"qT{ci}") for ci in range(2)]
        v_sb = pp.tile([128, NM, 256], f32r, tag="v", name="v_sb")
        zT = [pp.tile([128, HALF], f32, tag=f"zT{ci}", name=f"zT{ci}") for ci in range(2)]
        pe = [pp.tile([128, HALF], f32, tag=f"pe{ci}", name=f"pe{ci}") for ci in range(2)]

        # ---- phase 0: transposes + linear layers ----
        with (
            tc.tile_pool(name="xpool", bufs=1) as xp,
            tc.tile_pool(name="ldpool", bufs=4) as ld,
        ):
            xT = [xp.tile([128, N], f32r, tag=f"xT{ci}", name=f"xT{ci}") for ci in range(2)]
            xqT = [xp.tile([128, NQW], f32r, tag=f"xqT{ci}", name=f"xqT{ci}") for ci in range(2)]

            def transpose_in(dst, src, n_rows):
                for t in range(n_rows // 128):
                    xt = ld.tile([128, 256], f32, tag="ld", name="ld")
                    nc.sync.dma_start(xt[:, :], src[128 * t:128 * (t + 1), :])
                    for ci in range(2):
                        tp = ps.tile([128, 128], f32, tag="tpx", name="tp", bufs=3)
                        nc.tensor.transpose(
                            tp[:, :], xt[:, 128 * ci:128 * (ci + 1)], ident[:, :])
                        nc.vector.tensor_copy(
                            dst[ci][:, 128 * t:128 * (t + 1)], tp[:, :])

            transpose_in(xT, X, N)
            transpose_in(xqT, XQ, NQW)

            # k^T [c_out, m] = W_k.T @ x^T ; q^T likewise from xq^T
            def linT(dst, w_sb, src, n_cols, copy_eng):
                nslices = (n_cols + 511) // 512
                for co in range(2):
                    for s in range(nslices):
                        c0, c1 = 512 * s, min(512 * (s + 1), n_cols)
                        pm = ps.tile([128, 512], f32, tag="mm", name="pm", bufs=3)
                        for ci in range(2):
                            nc.tensor.matmul(
                                pm[:, :c1 - c0],
                                _r(w_sb[ci][:, 128 * co:128 * (co + 1)]),
                                _r(src[ci][:, c0:c1]),
                                start=(ci == 0), stop=(ci == 1))
                        copy_eng(dst[co][:, c0:c1], pm[:, :c1 - c0])

            linT(kT, wk_sb, xT, N, nc.scalar.copy)
            linT(qT, wq_sb, xqT, NQW, nc.scalar.copy)

            # v natural: x^T chunks stationary
            for m in range(NM):
                pm = ps.tile([128, 256], f32, tag="mm", name="pm", bufs=3)
                for ci in range(2):
                    nc.tensor.matmul(
                        pm[:, :], _r(xT[ci][:, 128 * m:128 * (m + 1)]),
                        _r(wv_sb[ci]), start=(ci == 0), stop=(ci == 1))
                nc.scalar.copy(v_sb[:, m, :], pm[:, :])

            # z^T = gelu(W_g.T @ xq^T[own half] + bg)
            for co in range(2):
                for g in range(NG):
                    c0 = 512 * g
                    pm = ps.tile([128, 512], f32, tag="mm", name="pm", bufs=3)
                    for ci in range(2):
                        nc.tensor.matmul(
                            pm[:, :],
                            _r(wg_sb[ci][:, 128 * co:128 * (co + 1)]),
                            _r(xqT[ci][:, HALO + c0:HALO + c0 + 512]),
                            start=(ci == 0), stop=(ci == 1))
                    nc.scalar.activation(
                        zT[co][:, c0:c0 + 512], pm[:, :],
                        mybir.ActivationFunctionType.Gelu,
                        bias=bg_sb[:, co:co + 1], scale=1.0)

        with tc.tile_pool(name="attn", bufs=1) as at:
            # ---- pe depthwise conv on q^T (9 fused taps per chunk) ----
            for co in range(2):
                q3 = qT[co][:, :].bitcast(f32).rearrange("p (r c) -> p r c", c=WS)
                p3 = pe[co][:, :].rearrange("p (r c) -> p r c", c=WS)
                # center tap (a=1,b=1) initializes acc with bias
                nc.vector.tensor_scalar(
                    pe[co][:, :], qT[co][:, HALO:HALO + HALF].bitcast(f32),
                    pw_sb[:, co, 4:5], pb_sb[:, co:co + 1],
                    mybir.AluOpType.mult, mybir.AluOpType.add)
                for a in range(3):
                    for b in range(3):
                        if a == 1 and b == 1:
                            continue
                        wlo = max(0, 1 - b)
                        whi = WS - max(0, b - 1)
                        nc.vector.scalar_tensor_tensor(
                            p3[:, :, wlo:whi],
                            q3[:, a:a + 32, wlo + b - 1:whi + b - 1],
                            pw_sb[:, co, 3 * a + b:3 * a + b + 1],
                            p3[:, :, wlo:whi],
                            mybir.AluOpType.mult, mybir.AluOpType.add)

            # ---- attention ----
            for g in range(NG):
                qs = HALO + 512 * g
                ot = [ps.tile([128, 512], f32, tag="ot", name="ot", bufs=2) for _ in range(2)]
                sums = ps.tile([1, 512], f32, tag="tpx", name="sums", bufs=3)
                for m in range(NM):
                    st = ps.tile([128, 512], f32, tag="mm", name="st", bufs=3)
                    for ci in range(2):
                        nc.tensor.matmul(
                            st[:, :], _r(kT[ci][:, 128 * m:128 * (m + 1)]),
                            _r(qT[ci][:, qs:qs + 512]),
                            start=(ci == 0), stop=(ci == 1))
                    p = at.tile([128, 512], f32r, tag="p", name="p", bufs=3)
                    nc.scalar.activation(
                        p[:, :], st[:, :],
                        mybir.ActivationFunctionType.Exp, scale=float(SCALE))
                    for ch in range(2):
                        nc.tensor.matmul(
                            ot[ch][:, :], _r(v_sb[:, m, 128 * ch:128 * (ch + 1)]),
                            _r(p[:, :]),
                            start=(m == 0), stop=(m == NM - 1))
                    nc.tensor.matmul(
                        sums[:, :], _r(ones_m[:, :]), _r(p[:, :]),
                        start=(m == 0), stop=(m == NM - 1))

                recip = at.tile([1, 512], f32, tag="recip", name="recip", bufs=2)
                nc.vector.reciprocal(recip[:, :], sums[:, :])
                rb_ps = ps.tile([128, 512], f32, tag="mm", name="rb_ps", bufs=3)
                nc.tensor.matmul(rb_ps[:, :], ones_1[:, :], recip[:, :],
                                 start=True, stop=True)
                rb = at.tile([128, 512], f32, tag="rb", name="rb", bufs=2)
                nc.scalar.copy(rb[:, :], rb_ps[:, :])

                mg = [at.tile([128, 512], f32r, tag="mg", name="mg", bufs=4) for _ in range(2)]
                for ch in range(2):
                    c0 = 512 * g
                    nc.vector.tensor_mul(mg[ch][:, :], ot[ch][:, :], rb[:, :])
                    nc.vector.tensor_add(
                        mg[ch][:, :], mg[ch][:, :], pe[ch][:, c0:c0 + 512])
                    nc.vector.tensor_mul(
                        mg[ch][:, :], mg[ch][:, :], zT[ch][:, c0:c0 + 512])

                for nt in range(4):
                    pj = ps.tile([128, 256], f32, tag="tpx", name="pj", bufs=3)
                    for ch in range(2):
                        nc.tensor.matmul(
                            pj[:, :], _r(mg[ch][:, 128 * nt:128 * (nt + 1)]),
                            _r(wp_sb[ch]), start=(ch == 0), stop=(ch == 1))
                    yt = at.tile([128, 256], f32, tag="y", name="yt", bufs=3)
                    nc.vector.tensor_copy(yt[:, :], pj[:, :])
                    r0 = 512 * g + 128 * nt
                    nc.sync.dma_start(Y[r0:r0 + 128, :], yt[:, :])


def build():
    global _nc_cache
    if _nc_cache is not None:
        return _nc_cache
    nc = bacc.Bacc("TRN2", target_bir_lowering=False, debug=False,
                   num_devices=N_CORES)
    X = nc.dram_tensor("x", [N, C], f32, kind="ExternalInput").ap()
    XQ = nc.dram_tensor("xq", [NQW, C], f32, kind="ExternalInput").ap()
    WQ = nc.dram_tensor("wq", [C, C], f32, kind="ExternalInput").ap()
    WK = nc.dram_tensor("wk", [C, C], f32, kind="ExternalInput").ap()
    WV = nc.dram_tensor("wv", [C, C], f32, kind="ExternalInput").ap()
    WG = nc.dram_tensor("wg", [C, C], f32, kind="ExternalInput").ap()
    BG = nc.dram_tensor("bg", [C, 1], f32, kind="ExternalInput").ap()
    WP = nc.dram_tensor("wp", [C, C], f32, kind="ExternalInput").ap()
    PW = nc.dram_tensor("pw", [C, 9], f32, kind="ExternalInput").ap()
    PB = nc.dram_tensor("pb", [C, 1], f32, kind="ExternalInput").ap()
    IDN = nc.dram_tensor("ident", [128, 128], f32, kind="ExternalInput").ap()
    Y = nc.dram_tensor("y", [HALF, C], f32, kind="ExternalOutput").ap()
    with tile.TileContext(nc) as tc:
        _emit(nc, tc, (X, XQ, WQ, WK, WV, WG, BG, WP, PW, PB, IDN, Y))
    nc.compile()
    _nc_cache = nc
    return nc


def make_in_maps(x, W_qkv, W_gate, b_gate, W_proj, pe_w, pe_b):
    x = np.asarray(x, np.float32)
    W_qkv = np.asarray(W_qkv, np.float32)
    wq = np.ascontiguousarray(W_qkv[:, :C])
    wk = np.ascontiguousarray(W_qkv[:, C:2 * C])
    wv = np.ascontiguousarray(W_qkv[:, 2 * C:])
    wg = np.ascontiguousarray(np.asarray(W_gate, np.float32))
    bg = np.asarray(b_gate, np.float32).reshape(C, 1).copy()
    wp = np.ascontiguousarray(np.asarray(W_proj, np.float32))
    pw = np.asarray(pe_w, np.float32).reshape(C, 9).copy()
    pb = np.asarray(pe_b, np.float32).reshape(C, 1).copy()
    ident = np.eye(128, dtype=np.float32)

    in_maps = []
    for core in range(N_CORES):
        b, h = divmod(core, 2)
        xb = np.ascontiguousarray(x[b])
        s, e = h * HALF, (h + 1) * HALF
        xq = np.zeros((NQW, C), np.float32)
        lo, hi = max(0, s - HALO), min(N, e + HALO)
        xq[lo - (s - HALO):hi - (s - HALO)] = xb[lo:hi]
        in_maps.append(dict(x=xb, xq=xq, wq=wq, wk=wk, wv=wv, wg=wg, bg=bg,
                            wp=wp, pw=pw, pb=pb, ident=ident))
    return in_maps


def run(in_maps, **kw):
    nc = build()
    return run_bass_kernel_spmd(nc, in_maps, list(range(N_CORES)), **kw)


def kernel(x, W_qkv, W_gate, b_gate, W_proj, pe_w, pe_b):
    res = run(make_in_maps(x, W_qkv, W_gate, b_gate, W_proj, pe_w, pe_b))
    y = np.empty((B, N, C), np.float32)
    for core in range(N_CORES):
        b, h = divmod(core, 2)
        y[b, h * HALF:(h + 1) * HALF] = res.results[core]["y"]
    return y
